# revision 5
# baseline (speedup 1.0000x reference)
# Mixture-of-two-experts (modality-routed) token GEMM on 8 Trainium2 NeuronCores.
#
# reference computes BOTH expert GEMMs and selects per token; only one GEMM per
# token is needed. Strategy (expert-dispatch, per the sharding hint):
#   host: partition tokens by type_id, pad each expert's token list to
#         4 * n_tok rows, transpose to [D, n_tok] per core chunk.
#   device (SPMD, uniform program): plain GEMM  y[tok, e] = x[tok, :] @ Wt + b
#         Cores 0-3 carry expert-0 tokens + W0, cores 4-7 expert-1 tokens + W1
#         (weights arrive as data, so the per-core program is identical).
#   host: inverse-scatter per-expert outputs back to [B, S, D] fp32.
#
# Mixed precision: contraction k 0..1535 runs fp16 (PE streams 1 col/cycle);
# k 1536..2047 runs as two fp8e4m3 DoubleRow pairs (2 k-planes per PE cell,
# 2 MACs/cycle) - 12.5% fewer PE cycles. Measured end-to-end rel err 1.63e-2
# (fp8 quantization noise over 1/4 of the contraction), inside the 2e-2 gate.
# Scales: x8 = x*16, w8 = W*64, fp16 W pre-scaled *1024 so one PSUM chain is
# consistent at 1024*y; the host divides the fp16 output by 1024.

import os
import sys
import time

import numpy as np
import ml_dtypes

for _p in ("/opt/trn_rl_repo", "/root/.axon_site/_ro/trn_rl_repo"):
    if os.path.isdir(_p) and _p not in sys.path:
        sys.path.insert(0, _p)

import concourse.bacc as bacc
import concourse.mybir as mybir
import concourse.tile as tile
from concourse.bass_utils import run_bass_kernel_spmd

D = 2048
KT = D // 128  # 16 contraction tiles
K16 = 12  # fp16 k-tiles (k 0..1535)
NPAIR = 2  # fp8 DoubleRow pairs (k 1536..2047, 256 contraction rows each)
KSPLIT = K16 * 128
SX = 16.0  # fp8 scale on x
SW = 64.0  # fp8 scale on W
SCALE = SX * SW  # PSUM carries SCALE*y; host divides it out
N_CORES = 8
CORES_PER_EXPERT = 4
F8 = ml_dtypes.float8_e4m3fn

_PROGRAM_CACHE: dict[int, object] = {}
LAST_RESULTS = None  # BassKernelResults of the most recent launch (for profiling)


def _build_program(n_tok: int):
    """One NeuronCore program: y[n_tok, D] = SCALE * (x @ W.T) + SCALE*bias.

    fp16 operands for k<KSPLIT, fp8e4 DoubleRow pairs above; fp32 PSUM; fp16 out.
    """
    m_tiles = n_tok // 128
    f16 = mybir.dt.float16
    f32 = mybir.dt.float32
    f8 = mybir.dt.float8e4
    DR = mybir.MatmulPerfMode.DoubleRow

    nc = bacc.Bacc("TRN2", target_bir_lowering=False, debug=False, num_devices=N_CORES)
    xt = nc.dram_tensor("xt", [K16, 128, n_tok], f16, kind="ExternalInput").ap()
    xt8 = nc.dram_tensor("xt8", [NPAIR, 128, 2, n_tok], f8, kind="ExternalInput").ap()
    wt = nc.dram_tensor("wt", [K16, 128, D], f16, kind="ExternalInput").ap()
    wt8 = nc.dram_tensor("wt8", [NPAIR, 128, 2, D], f8, kind="ExternalInput").ap()
    bias = nc.dram_tensor("bias", [128, D], f16, kind="ExternalInput").ap()
    y = nc.dram_tensor("y", [n_tok, D], f16, kind="ExternalOutput").ap()
    y_t = y.rearrange("(m p) e -> m p e", p=128)

    # The PE can only keep 2 full-width PSUM accumulation chains in flight, so
    # during the ~40us operand load it starves between k-tile arrivals. Fix:
    # the first N_SPLIT m-tiles accumulate the first 8 fp16 k-tiles into SBUF
    # partials as soon as that half lands (phase A), and finish the rest
    # (4 fp16 k-tiles + 2 fp8 pairs) later (phase B). Everything else runs the
    # plain full-k walk.
    n_split = 5 if m_tiles >= 8 else 0

    # k-units: ('f', k) = one fp16 k-tile, ('d', j) = one fp8 DoubleRow pair.
    units_a = [("f", k) for k in range(8)]
    units_b = [("f", k) for k in range(8, K16)] + [("d", j) for j in range(NPAIR)]
    units_full = units_a + units_b

    with tile.TileContext(nc) as tc:
        with (
            tc.tile_pool(name="wp", bufs=1) as wp,
            tc.tile_pool(name="xp", bufs=1) as xp,
            tc.tile_pool(name="bp", bufs=1) as bp,
            tc.tile_pool(name="ap", bufs=1) as apool,
            tc.tile_pool(name="op", bufs=3) as op_,
            tc.tile_pool(name="o32", bufs=2) as o32_,
            tc.tile_pool(name="pp", bufs=2, space="PSUM") as pp,
        ):
            # Whole operand set fits in SBUF; per-k tiles so matmuls start as
            # soon as the first slices land. Bias is loaded late - it is first
            # needed at the phase-B drains.
            # x tiles are split at the phase-A/B boundary: the "head" columns
            # (m-tiles 0..n_split-1) are what the early split-k chains need,
            # so loading all heads+weights first matches the PE's demand
            # during the load window. Tails follow; they are only needed by
            # the full m-tile walks that start much later.
            head = n_split * 128
            tail = n_tok - head
            xh, wk = [], []
            bias_s = bp.tile([128, D], f16, name="bias_s")
            # single HWDGE ring: FIFO transfer order doubles as a priority
            # scheme - (w,xh) pairs for phase A first, then phase-B operands
            # (fp16 k 8..11, then the fp8 pairs), bias, tails last.
            for k in range(K16):
                ws = wp.tile([128, D], f16, name=f"w{k}", tag=f"w{k}")
                if k < 4:
                    # finer arrival granularity during the DMA ramp: matmuls
                    # on the first two output chunks can start before the
                    # full weight tile lands (Tile deps are range-based)
                    nc.sync.dma_start(ws[:, 0 : D // 2], wt[k][:, 0 : D // 2])
                    nc.sync.dma_start(ws[:, D // 2 : D], wt[k][:, D // 2 : D])
                else:
                    nc.sync.dma_start(ws[:], wt[k])
                wk.append(ws)
                if n_split:
                    h = xp.tile([128, head], f16, name=f"xh{k}", tag=f"xh{k}")
                    nc.sync.dma_start(h[:], xt[k][:, 0:head])
                    xh.append(h)
            w8t, x8t = [], []
            for j in range(NPAIR):
                w8 = wp.tile([128, 2, D], f8, name=f"w8_{j}", tag=f"w8_{j}")
                nc.sync.dma_start(w8[:], wt8[j])
                w8t.append(w8)
                x8 = xp.tile([128, 2, n_tok], f8, name=f"x8_{j}", tag=f"x8_{j}")
                nc.sync.dma_start(x8[:], xt8[j])
                x8t.append(x8)
            nc.sync.dma_start(bias_s[:], bias[:])
            # tails: only needed by the late full-k walks; batch 4 k-tiles
            # per transfer to cut issue count
            xtl = []
            xt_r = xt.rearrange("(g k) p n -> g k p n", k=4)
            for g in range(K16 // 4):
                t = xp.tile([128, 4, tail], f16, name=f"xt{g}", tag=f"xt{g}")
                nc.sync.dma_start(t[:], xt_r[g][:, :, head:n_tok].rearrange("k p n -> p k n"))
                xtl.append(t)

            def lhs_slice(k, m):
                if m < n_split:
                    return xh[k][:, m * 128 : (m + 1) * 128]
                j = m - n_split
                return xtl[k // 4][:, k % 4, j * 128 : (j + 1) * 128]

            # PE warm-up: matmuls on a zeroed tile, no DMA dependency. Runs
            # during the DMA ramp (PE would idle anyway) and flips the HAM
            # clock gate to 8/8 before the first real matmul. memset on DVE:
            # it boots by ~4.7us and memsets in ~200ns, so the warm-up starts
            # ~2us earlier than with the gpsimd memset.
            wz = bp.tile([128, 512], f16, name="wz")
            nc.vector.memset(wz[:], 0.0)
            psw = pp.tile([128, 512], f32, name="psw", tag="ps")
            for _ in range(14):
                nc.tensor.matmul(psw[:], wz[:, 0:128], wz[:], start=True, stop=True)

            def unit_mm(ps, m, u, c, start, stop):
                if u[0] == "f":
                    return nc.tensor.matmul(
                        ps[:, c * 512 : (c + 1) * 512],
                        lhs_slice(u[1], m),
                        wk[u[1]][:, c * 512 : (c + 1) * 512],
                        start=start,
                        stop=stop,
                    )
                j = u[1]
                return nc.tensor.matmul(
                    ps[:, c * 512 : (c + 1) * 512],
                    x8t[j][:, :, m * 128 : (m + 1) * 128],
                    w8t[j][:, :, c * 512 : (c + 1) * 512],
                    start=start,
                    stop=stop,
                    perf_mode=DR,
                )

            def mm_chain(ps, m, units):
                first = last = None
                for i, u in enumerate(units):
                    for c in range(4):
                        mm = unit_mm(ps, m, u, c, i == 0, i == len(units) - 1)
                        first = first or mm
                        last = mm
                return first, last

            def drain(ps, m):
                # single full-width op: DVE reads PSUM across banks fine, and
                # fewer instructions -> fewer sems -> shorter end-of-kernel
                # semaphore-reset storm
                ot = op_.tile([128, D], f16, name=f"ot{m}", tag="ot")
                nc.vector.tensor_add(ot[:], ps[:], bias_s[:])
                nc.sync.dma_start(y_t[m], ot[:])

            prev_last = None

            def pin(first, reason):
                # keep the PE stream in emission order chain-by-chain: the
                # scheduler otherwise hoists later chains (gated on late k
                # arrivals) ahead of ready work and stalls the PE
                if prev_last is not None:
                    tile.add_dep_helper(
                        first.ins, prev_last.ins, sync=False, reason=reason
                    )

            # Pins enforce PHASE order only (all A before any B before any F):
            # chains WITHIN a phase stay unpinned so the scheduler can
            # interleave them - during the DMA ramp each arriving k-pair then
            # unlocks work from every in-flight chain, not just one.
            acc = {}
            a_lasts = []
            for m in range(n_split):  # phase A: fp16 k=0..7 -> SBUF partial
                ps = pp.tile([128, D], f32, name=f"psa{m}", tag="ps")
                fa, la = mm_chain(ps, m, units_a)
                a_lasts.append(la)
                # no bias here: bias must not gate the phase-A psum drains
                a = apool.tile([128, D], f32, name=f"acc{m}", tag=f"acc{m}")
                nc.vector.tensor_copy(a[:], ps[:])
                acc[m] = a

            b_lasts = []
            for m in range(n_split):  # phase B: rest of k + partial + bias
                ps = pp.tile([128, D], f32, name=f"psb{m}", tag="ps")
                fb, lb = mm_chain(ps, m, units_b)
                for la in a_lasts:
                    tile.add_dep_helper(fb.ins, la.ins, sync=False, reason="A->B")
                b_lasts.append(lb)
                ot32 = o32_.tile([128, D], f32, name=f"otb32_{m}", tag="ot32")
                nc.vector.tensor_add(ot32[:], ps[:], acc[m][:])
                ot = op_.tile([128, D], f16, name=f"otb{m}", tag="ot")
                nc.vector.tensor_add(ot[:], ot32[:], bias_s[:])
                nc.sync.dma_start(y_t[m], ot[:])
            prev_last = None

            full = list(range(n_split, m_tiles))
            for m in full[:-1]:
                ps = pp.tile([128, D], f32, name=f"ps{m}", tag="ps")
                ff, lf = mm_chain(ps, m, units_full)
                if m == full[0]:
                    for lb in b_lasts:
                        tile.add_dep_helper(ff.ins, lb.ins, sync=False, reason="B->F")
                else:
                    pin(ff, f"chain order F{m}")
                prev_last = lf
                drain(ps, m)

            # last m-tile: accumulate one 512-wide chunk at a time so chunk c
            # drains (DVE add + DMA) while chunk c+1's matmuls still run -
            # shortens the kernel tail after the final matmul. Each chunk gets
            # its OWN psum tile (same tag -> alternates the 2 slots): chunks
            # in one tile serialize ~800ns each on accumulation-group
            # tracking, separate tiles pipeline cleanly.
            m = full[-1]
            ot = op_.tile([128, D], f16, name=f"ot{m}", tag="ot")
            for c in range(4):
                sl = slice(c * 512, (c + 1) * 512)
                ps = pp.tile([128, 512], f32, name=f"psl{c}", tag="ps")
                for i, u in enumerate(units_full):
                    if u[0] == "f":
                        mm = nc.tensor.matmul(
                            ps[:],
                            lhs_slice(u[1], m),
                            wk[u[1]][:, sl],
                            start=(i == 0),
                            stop=(i == len(units_full) - 1),
                        )
                    else:
                        mm = nc.tensor.matmul(
                            ps[:],
                            x8t[u[1]][:, :, m * 128 : (m + 1) * 128],
                            w8t[u[1]][:, :, sl],
                            start=(i == 0),
                            stop=(i == len(units_full) - 1),
                            perf_mode=DR,
                        )
                    if i == 0 and c == 0:
                        pin(mm, f"chain order F{m}")
                nc.vector.tensor_add(ot[:, sl], ps[:], bias_s[:, sl])
                nc.sync.dma_start(y_t[m][:, sl], ot[:, sl])

    nc.compile()
    return nc


def _get_program(n_tok: int):
    if n_tok not in _PROGRAM_CACHE:
        _PROGRAM_CACHE[n_tok] = _build_program(n_tok)
    return _PROGRAM_CACHE[n_tok]


def _round_up(v: int, m: int) -> int:
    return -(-v // m) * m


def _q8(a: np.ndarray, scale: float) -> np.ndarray:
    return np.clip(a * scale, -240.0, 240.0).astype(F8)


def kernel(hidden_states, type_ids, W0, b0, W1, b1, _trace=False, _tmpdir=None):
    global LAST_RESULTS

    B, S, D_ = hidden_states.shape
    assert D_ == D
    x = np.ascontiguousarray(np.asarray(hidden_states, dtype=np.float32)).reshape(
        B * S, D
    )
    t = np.asarray(type_ids).reshape(B * S)

    idx = [np.nonzero(t == e)[0] for e in (0, 1)]
    counts = [len(i) for i in idx]
    # tokens per core: 4 cores per expert, padded to 128-token tiles.
    # n_tok is SBUF-limited; extremely skewed expert splits fall back to
    # multiple launches of the same program over token slices.
    N_TOK_MAX = 2176  # largest n_tok whose operand set fits in SBUF
    n_tok = max(128, _round_up(-(-max(counts) // CORES_PER_EXPERT), 128))
    n_tok = min(n_tok, N_TOK_MAX)
    cap = n_tok * CORES_PER_EXPERT
    n_launches = -(-max(counts) // cap)

    nc = _get_program(n_tok)

    wts, wt8s, biases = [], [], []
    for W, b in ((W0, b0), (W1, b1)):
        WT = np.asarray(W, dtype=np.float32).T  # [d, e]
        wts.append(
            np.ascontiguousarray((WT[:KSPLIT] * SCALE).astype(np.float16)).reshape(
                K16, 128, D
            )
        )
        # pair j, plane i, partition p  <->  contraction row KSPLIT+256j+128i+p
        wt8s.append(
            np.ascontiguousarray(
                _q8(WT[KSPLIT:], SW).reshape(NPAIR, 2, 128, D).transpose(0, 2, 1, 3)
            )
        )
        biases.append(
            np.ascontiguousarray(
                np.broadcast_to(
                    (np.asarray(b, dtype=np.float32) * SCALE).astype(np.float16),
                    (128, D),
                )
            )
        )

    gathered = [x[idx[e]] for e in (0, 1)]  # [count_e, D] fp32

    out = np.empty((B * S, D), dtype=np.float32)
    parts = [[], []]
    for li in range(n_launches):
        in_maps = []
        for e in (0, 1):
            g = gathered[e][li * cap : (li + 1) * cap]
            if g.shape[0] < cap:
                g = np.concatenate(
                    [g, np.zeros((cap - g.shape[0], D), np.float32)], axis=0
                )
            for c in range(CORES_PER_EXPERT):
                chunk = g[c * n_tok : (c + 1) * n_tok]  # [n_tok, D] fp32
                ct = chunk.T  # [D, n_tok]
                xt_c = np.ascontiguousarray(ct[:KSPLIT].astype(np.float16)).reshape(
                    K16, 128, n_tok
                )
                xt8_c = np.ascontiguousarray(
                    _q8(ct[KSPLIT:], SX)
                    .reshape(NPAIR, 2, 128, n_tok)
                    .transpose(0, 2, 1, 3)
                )
                in_maps.append(
                    {
                        "xt": xt_c,
                        "xt8": xt8_c,
                        "wt": wts[e],
                        "wt8": wt8s[e],
                        "bias": biases[e],
                    }
                )

        res = None
        for attempt in range(3):
            try:
                res = run_bass_kernel_spmd(
                    nc, in_maps, list(range(N_CORES)), trace=_trace, tmpdir=_tmpdir
                )
                break
            except Exception:
                # transient NRT_EXEC_UNIT_UNRECOVERABLE has been observed when
                # a run starts right as a previous process tears the device down
                if attempt == 2:
                    raise
                time.sleep(10)
        LAST_RESULTS = res
        for e in (0, 1):
            parts[e].extend(
                res.results[e * CORES_PER_EXPERT + c]["y"]
                for c in range(CORES_PER_EXPERT)
            )

    inv = np.float32(1.0 / SCALE)
    for e in (0, 1):
        full_e = np.concatenate(parts[e], axis=0)[: counts[e]]
        out[idx[e]] = full_e.astype(np.float32) * inv
    return out.reshape(B, S, D)
